# revision 20
# baseline (speedup 1.0000x reference)
"""Trainium2 Bass kernel for nn_Attention (dense transformer block).

Computes, for x [2, 256, 64, 64]:
  qkv = BN(1x1conv(x));  q,k,v per 8 heads (kd=16, hd=32)
  attn = softmax(q^T k * kd^-0.5); out = v @ attn^T
  pe   = BN(depthwise3x3(v))
  y    = BN(1x1conv(out + pe))

Sharding: spatial (N = H*W = 4096) split 8 ways -> 512 columns per core
for both batch elements. Each core redundantly computes full k / v^T
(needed for its attention columns); q / pe / proj only for its shard.
No collectives.

Layout choices:
  - scores computed transposed: S^T[m, n] (m on partitions) so the
    softmax denominator comes from a ones-column in the weights of the
    numerator matmul (rows of softmax sum over partitions).
  - exp has no max-subtraction (scores are O(1) here; fp32 exp safe).
  - BN scale folded into weights host-side; bias via per-partition adds.
    v's BN bias commutes through softmax (rows sum to 1) and is added at
    the end together with pe's bias.
  - matmul operands in bf16 (PE full rate; accumulation stays fp32 in
    PSUM). The q/k channel dim is zero-padded 16->32 so every head's
    rows start at a 32-aligned partition (matmul base requirement).
"""

import numpy as np

# ---- problem constants (hardcoded; harness provides only the inputs) ----
B = 2
C = 256
H = W = 64
N = H * W                      # 4096
NH = 8
KD = 16
HD = 32
SCALE = KD ** -0.5             # 0.25
BN_EPS = 1e-3
NCORES = 8
NS = N // NCORES               # 512 shard columns per core per batch
RS = NS // W                   # 8 image rows per shard
MCH = N // 128                 # 32 m-chunks of 128
GSZ = 3                        # S^T psum group size (3 banks per buffer)

_CACHE = {}


def _patch_tail_drain(tile_mod, mybir):
    """This toolchain's walrus rejects >1 sync wait per instruction; Tile's
    kernel-tail drain accumulates one wait per active proc. Split them
    across single-wait nops."""
    from concourse.tile import ScopedClock

    def _drain_and_barrier(self, tick_clock, wait_clock):
        nop_inst = self.nc.sync.nop(nofuse=True)
        wait_clock.add_sem_waits(
            nop_inst.ins, ScopedClock({None: tick_clock.global_clock})
        )
        si = nop_inst.ins.sync_info
        waits = list(si.on_wait) if si is not None else []
        if len(waits) > 1:
            si.on_wait = [waits[0]]
            for w in waits[1:]:
                extra = self.nc.sync.nop(nofuse=True)
                extra.ins.sync_info = mybir.SyncInfo(on_wait=[w], on_update=[])
        self.nc.sync.drain()
        self.nc.all_engine_barrier()
        assert self.sems is not None
        popped = self.nc._tile_sem_poison_stack.pop()
        assert popped is self._sem_poison
        self.nc.clear_and_free_semaphores(list(self.sems.allocated().values()))
        self.nc.all_engine_barrier()

    tile_mod.TileContext._drain_and_barrier = _drain_and_barrier


def _split_multi_waits(nc, mybir):
    """Walrus in this toolchain accepts at most one sync wait per
    instruction; hoist extra waits onto single-wait nops inserted just
    before the instruction on the same engine (in-order engines, so
    waiting earlier is semantics-preserving)."""
    idx = 0
    for f in nc.m.functions:
        for bb in f.blocks:
            il = bb.instructions
            if not any(
                inst.sync_info is not None and len(inst.sync_info.on_wait) > 1
                for inst in il
            ):
                continue
            new = []
            for inst in il:
                si = inst.sync_info
                if si is not None and len(si.on_wait) > 1:
                    waits = list(si.on_wait)
                    for w in waits[:-1]:
                        nop = mybir.InstNoOp(name=f"wsplit-{idx}", ins=[], outs=[])
                        idx += 1
                        nop.engine = inst.engine
                        nop.sync_info = mybir.SyncInfo(on_wait=[w], on_update=[])
                        new.append(nop)
                    si.on_wait = [waits[-1]]
                new.append(inst)
            bb.instructions = new


def build_module():
    """Build the (shard-agnostic) single-core Bass module run SPMD on 8 cores."""
    import contextlib

    import concourse.bass as bass
    import concourse.tile as tile
    from concourse import mybir

    _patch_tail_drain(tile, mybir)

    f32 = mybir.dt.float32
    bf16 = mybir.dt.bfloat16

    nc = bass.Bass()

    # -------- dram parameters --------
    x_ext = nc.declare_dram_parameter("x", [B, C, N], f32, isOutput=False)
    xq_ext = nc.declare_dram_parameter("xq", [B, C, NS], f32, isOutput=False)
    xh_ext = nc.declare_dram_parameter("xh", [B, C, (RS + 2) * W], f32, isOutput=False)
    hm_ext = nc.declare_dram_parameter("hmask", [128, (RS + 2) * 66], f32, isOutput=False)
    wq_ext = nc.declare_dram_parameter("wq_t", [C, 256], f32, isOutput=False)
    bq_ext = nc.declare_dram_parameter("bq", [256, 1], f32, isOutput=False)
    wk_ext = nc.declare_dram_parameter("wk_t", [C, 256], f32, isOutput=False)
    bk_ext = nc.declare_dram_parameter("bk", [256, 1], f32, isOutput=False)
    wv_ext = nc.declare_dram_parameter("wv_t", [C, C], f32, isOutput=False)
    bv_ext = nc.declare_dram_parameter("bv", [C, 1], f32, isOutput=False)
    wpe_ext = nc.declare_dram_parameter("wpe", [C, 9], f32, isOutput=False)
    bvpe_ext = nc.declare_dram_parameter("bvpe", [C, 1], f32, isOutput=False)
    wp_ext = nc.declare_dram_parameter("wp_t", [C, C], f32, isOutput=False)
    bp_ext = nc.declare_dram_parameter("bp", [C, 1], f32, isOutput=False)
    y_ext = nc.declare_dram_parameter("y", [B, C, NS], f32, isOutput=True)

    Exp = mybir.ActivationFunctionType.Exp

    with tile.TileContext(nc) as tc, contextlib.ExitStack() as ctx:
        consts = ctx.enter_context(tc.tile_pool(name="consts", bufs=1))
        stage = ctx.enter_context(tc.tile_pool(name="stage", bufs=1))
        perb1 = ctx.enter_context(tc.tile_pool(name="perb1", bufs=1))
        perb2 = ctx.enter_context(tc.tile_pool(name="perb2", bufs=2))
        epool = ctx.enter_context(tc.tile_pool(name="epool", bufs=3))
        small = ctx.enter_context(tc.tile_pool(name="small", bufs=2))
        ps_big = ctx.enter_context(tc.tile_pool(name="ps_big", bufs=2, space="PSUM"))
        ps_num = ctx.enter_context(tc.tile_pool(name="ps_num", bufs=2, space="PSUM"))

        # -------- load + bf16-convert weights (once) --------
        def load_bf16(name, ext, shape, rearr):
            st = stage.tile(shape, f32, tag="wstage")
            nc.sync.dma_start(out=st[:], in_=ext.rearrange(rearr, p=128))
            bft = consts.tile(shape, bf16, tag=name)
            nc.vector.tensor_copy(out=bft[:], in_=st[:])
            return bft

        wq_sb = load_bf16("wq", wq_ext, [128, 2, 256], "(c p) q -> p c q")
        wk_sb = load_bf16("wk", wk_ext, [128, 2, 256], "(c p) q -> p c q")
        wv_sb = load_bf16("wv", wv_ext, [128, 2, C], "(c p) v -> p c v")
        wp_sb = load_bf16("wp", wp_ext, [128, 2, C], "(c p) o -> p c o")

        def load_f32(name, ext, shape, rearr=None, **kw):
            t = consts.tile(shape, f32, tag=name)
            src = ext.rearrange(rearr, **kw) if rearr else ext[:]
            nc.sync.dma_start(out=t[:], in_=src)
            return t

        bq_sb = load_f32("bq", bq_ext, [128, 2], "(c p) u -> p (c u)", p=128)
        bk_sb = load_f32("bk", bk_ext, [128, 2], "(c p) u -> p (c u)", p=128)
        bv_sb = load_f32("bv", bv_ext, [128, 2], "(o p) u -> p (o u)", p=128)
        bvpe_sb = load_f32("bvpe", bvpe_ext, [128, 2], "(o p) u -> p (o u)", p=128)
        bp_sb = load_f32("bp", bp_ext, [128, 2], "(o p) u -> p (o u)", p=128)
        wpe_sb = load_f32("wpe", wpe_ext, [128, 2, 9], "(o p) t -> p o t", p=128)
        hm_sb = load_f32("hm", hm_ext, [128, RS + 2, 66], "p (r w) -> p r w", w=66)

        ones_sb = consts.tile([1, HD], f32)
        nc.vector.memset(ones_sb[:], 1.0)

        # v^T with a ones-column per head: [m-part, m-chunk, head, 32v+1]
        vT = perb2.tile([128, MCH, NH, HD + 1], bf16, tag="vT")
        nc.vector.memset(vT[:, :, :, HD : HD + 1], 1.0)

        for b in range(B):
            # ---- load x (full), xq, xh; convert to bf16 ----
            x_st = stage.tile([128, 2, N], f32, tag="x_st")
            nc.sync.dma_start(
                out=x_st[:], in_=x_ext[b].rearrange("(c p) n -> p c n", p=128)
            )
            x_bf = perb2.tile([128, 2, N], bf16, tag="x_bf")
            nc.vector.tensor_copy(out=x_bf[:], in_=x_st[:])
            xq_st = stage.tile([128, 2, NS], f32, tag="xq_st")
            nc.sync.dma_start(
                out=xq_st[:], in_=xq_ext[b].rearrange("(c p) n -> p c n", p=128)
            )
            xq_bf = perb2.tile([128, 2, NS], bf16, tag="xq_bf")
            nc.vector.tensor_copy(out=xq_bf[:], in_=xq_st[:])
            xh_st = stage.tile([128, 2, (RS + 2) * W], f32, tag="xh_st")
            nc.sync.dma_start(
                out=xh_st[:], in_=xh_ext[b].rearrange("(c p) n -> p c n", p=128)
            )
            xh_bf = perb2.tile([128, 2, (RS + 2) * W], bf16, tag="xh_bf")
            nc.vector.tensor_copy(out=xh_bf[:], in_=xh_st[:])

            # ---- q for this shard: [128, half, NS]; half hh holds heads
            # 4*hh..4*hh+3 at 32-aligned row groups ----
            q_sb = perb2.tile([128, 2, NS], bf16, tag="q")
            for hh in range(2):
                ps_q = ps_big.tile([128, NS], f32, tag="ps_big")
                for cc in range(2):
                    nc.tensor.matmul(
                        ps_q[:],
                        wq_sb[:, cc, hh * 128 : (hh + 1) * 128],
                        xq_bf[:, cc, :],
                        start=(cc == 0),
                        stop=(cc == 1),
                    )
                nc.vector.tensor_scalar_add(
                    out=q_sb[:, hh, :], in0=ps_q[:], scalar1=bq_sb[:, hh : hh + 1]
                )

            # ---- k (full N): [128, half, N] ----
            k_sb = perb2.tile([128, 2, N], bf16, tag="k")
            for hh in range(2):
                for mt in range(N // 512):
                    ps_k = ps_big.tile([128, 512], f32, tag="ps_big")
                    for cc in range(2):
                        nc.tensor.matmul(
                            ps_k[:],
                            wk_sb[:, cc, hh * 128 : (hh + 1) * 128],
                            x_bf[:, cc, mt * 512 : (mt + 1) * 512],
                            start=(cc == 0),
                            stop=(cc == 1),
                        )
                    nc.vector.tensor_scalar_add(
                        out=k_sb[:, hh, mt * 512 : (mt + 1) * 512],
                        in0=ps_k[:],
                        scalar1=bk_sb[:, hh : hh + 1],
                    )

            # ---- v^T (full N), no bias: vT[m, h, d] ----
            for mc in range(MCH):
                ps_v = ps_big.tile([128, C], f32, tag="ps_big")
                for cc in range(2):
                    nc.tensor.matmul(
                        ps_v[:],
                        x_bf[:, cc, mc * 128 : (mc + 1) * 128],
                        wv_sb[:, cc, :],
                        start=(cc == 0),
                        stop=(cc == 1),
                    )
                nc.vector.tensor_copy(
                    out=vT[:, mc, :, 0:HD],
                    in_=ps_v[:].rearrange("p (h d) -> p h d", h=NH),
                )

            # ---- v on halo rows (with bias, masked) for pe conv ----
            vh = perb1.tile([128, 2, RS + 2, 66], f32, tag="vh")
            nc.vector.memset(vh[:], 0.0)
            for oc in range(2):
                for t in range(2):
                    ps_vh = ps_big.tile([128, (RS + 2) * W // 2], f32, tag="ps_big")
                    for cc in range(2):
                        nc.tensor.matmul(
                            ps_vh[:],
                            wv_sb[:, cc, oc * 128 : (oc + 1) * 128],
                            xh_bf[:, cc, t * 5 * W : (t + 1) * 5 * W],
                            start=(cc == 0),
                            stop=(cc == 1),
                        )
                    nc.vector.tensor_scalar_add(
                        out=vh[:, oc, t * 5 : (t + 1) * 5, 1 : 1 + W],
                        in0=ps_vh[:].rearrange("p (r w) -> p r w", w=W),
                        scalar1=bv_sb[:, oc : oc + 1],
                    )
                nc.vector.tensor_mul(out=vh[:, oc], in0=vh[:, oc], in1=hm_sb[:])

            # ---- pe depthwise 3x3 (no bias; bias folded into bvpe) ----
            pe_sb = perb1.tile([128, 2, RS, W], f32, tag="pe")
            for oc in range(2):
                for t in range(9):
                    dy, dx = t // 3, t % 3
                    tap = vh[:, oc, dy : dy + RS, dx : dx + W]
                    wt = wpe_sb[:, oc, t : t + 1]
                    if t == 0:
                        nc.vector.tensor_scalar_mul(
                            out=pe_sb[:, oc], in0=tap, scalar1=wt
                        )
                    else:
                        tmp = small.tile([128, RS, W], f32, tag="petmp")
                        nc.vector.tensor_scalar_mul(out=tmp[:], in0=tap, scalar1=wt)
                        nc.vector.tensor_add(
                            out=pe_sb[:, oc], in0=pe_sb[:, oc], in1=tmp[:]
                        )

            # ---- attention per head ----
            y_sb = perb1.tile([128, 2, NS], f32, tag="y")
            for h in range(NH):
                pn = ps_num.tile([HD + 1, NS], f32, tag="ps_num")
                hh, g32 = h // 4, (h % 4) * 32
                kh = k_sb[g32 : g32 + 32, hh, :]
                qh = q_sb[g32 : g32 + 32, hh, :]
                mc = 0
                while mc < MCH:
                    g = min(GSZ, MCH - mc)
                    ps_s = ps_big.tile([128, GSZ * NS], f32, tag="ps_big")
                    for j in range(g):
                        nc.tensor.matmul(
                            ps_s[:, j * NS : (j + 1) * NS],
                            kh[:, (mc + j) * 128 : (mc + j + 1) * 128],
                            qh,
                            start=True,
                            stop=True,
                            tile_position=(g32, 0),
                        )
                    e_sb = epool.tile([128, GSZ * NS], bf16, tag="E")
                    nc.scalar.activation(
                        out=e_sb[:, : g * NS],
                        in_=ps_s[:, : g * NS],
                        func=Exp,
                        scale=SCALE,
                    )
                    for j in range(g):
                        nc.tensor.matmul(
                            pn[:],
                            vT[:, mc + j, h, :],
                            e_sb[:, j * NS : (j + 1) * NS],
                            start=(mc + j == 0),
                            stop=(mc + j == MCH - 1),
                        )
                    mc += g
                # normalize: out rows = numer * (1/denom), denom = row HD.
                # Broadcast 1/denom across the 32 output partitions with a
                # K=1 fp32 ones-matmul (DMA can't partition-broadcast in SBUF).
                numer_sb = small.tile([HD + 1, NS], f32, tag="numer")
                nc.vector.tensor_copy(out=numer_sb[:], in_=pn[:])
                rec = small.tile([1, NS], f32, tag="rec")
                nc.vector.reciprocal(out=rec[:], in_=numer_sb[HD : HD + 1, :])
                rec_ps = ps_num.tile([HD, NS], f32, tag="ps_num")
                nc.tensor.matmul(
                    rec_ps[:], ones_sb[:], rec[:], start=True, stop=True
                )
                oc, row = h // 4, (h % 4) * HD
                nc.vector.tensor_mul(
                    out=y_sb[row : row + HD, oc, :],
                    in0=numer_sb[0:HD, :],
                    in1=rec_ps[:],
                )

            # ---- y = attn_out + (bv + bpe) + pe ; bf16 for proj ----
            y_bf = perb1.tile([128, 2, NS], bf16, tag="y_bf")
            o_sb = perb1.tile([128, 2, NS], f32, tag="o")
            for oc in range(2):
                nc.vector.tensor_scalar_add(
                    out=y_sb[:, oc, :],
                    in0=y_sb[:, oc, :],
                    scalar1=bvpe_sb[:, oc : oc + 1],
                )
                nc.vector.tensor_add(
                    out=y_bf[:, oc, :],
                    in0=y_sb[:, oc, :],
                    in1=pe_sb[:, oc].rearrange("p r w -> p (r w)"),
                )
            for oc in range(2):
                ps_p = ps_big.tile([128, NS], f32, tag="ps_big")
                for cc in range(2):
                    nc.tensor.matmul(
                        ps_p[:],
                        wp_sb[:, cc, oc * 128 : (oc + 1) * 128],
                        y_bf[:, cc, :],
                        start=(cc == 0),
                        stop=(cc == 1),
                    )
                nc.vector.tensor_scalar_add(
                    out=o_sb[:, oc, :], in0=ps_p[:], scalar1=bp_sb[:, oc : oc + 1]
                )
                nc.sync.dma_start(
                    out=y_ext[b, oc * 128 : (oc + 1) * 128, :],
                    in_=o_sb[:, oc, :],
                )

    return nc


def _prep_host(inputs):
    """Fold BN into weights; build per-core input maps."""
    x = np.ascontiguousarray(np.asarray(inputs["x"], dtype=np.float32))
    w_qkv = np.asarray(inputs["w_qkv"], dtype=np.float32)
    w_pe = np.asarray(inputs["w_pe"], dtype=np.float32)
    w_proj = np.asarray(inputs["w_proj"], dtype=np.float32)

    def fold(g, bta, m, v):
        s = np.asarray(g, np.float32) / np.sqrt(np.asarray(v, np.float32) + BN_EPS)
        return s, np.asarray(bta, np.float32) - np.asarray(m, np.float32) * s

    s_qkv, b_qkv = fold(inputs["qkv_g"], inputs["qkv_b"], inputs["qkv_m"], inputs["qkv_v"])
    s_pe, b_pe = fold(inputs["pe_g"], inputs["pe_b"], inputs["pe_m"], inputs["pe_v"])
    s_p, b_p = fold(inputs["proj_g"], inputs["proj_b"], inputs["proj_m"], inputs["proj_v"])

    wf = w_qkv * s_qkv[:, None]
    idx_v = np.concatenate([np.arange(h * 64 + 2 * KD, h * 64 + 64) for h in range(NH)])

    # q/k padded: channel h*32+kd holds head h's kd (kd<16); rest zero.
    wq_t = np.zeros((C, 256), np.float32)
    wk_t = np.zeros((C, 256), np.float32)
    bq = np.zeros((256, 1), np.float32)
    bk = np.zeros((256, 1), np.float32)
    for h in range(NH):
        wq_t[:, h * 32 : h * 32 + KD] = wf[h * 64 : h * 64 + KD].T
        wk_t[:, h * 32 : h * 32 + KD] = wf[h * 64 + KD : h * 64 + 2 * KD].T
        bq[h * 32 : h * 32 + KD, 0] = b_qkv[h * 64 : h * 64 + KD]
        bk[h * 32 : h * 32 + KD, 0] = b_qkv[h * 64 + KD : h * 64 + 2 * KD]

    wv_t = np.ascontiguousarray(wf[idx_v].T)            # [C, C]
    bv = np.ascontiguousarray(b_qkv[idx_v][:, None])
    wpe = np.ascontiguousarray((w_pe[:, 0] * s_pe[:, None, None]).reshape(C, 9))
    bvpe = np.ascontiguousarray((b_qkv[idx_v] + b_pe)[:, None])
    wp_t = np.ascontiguousarray((w_proj * s_p[:, None]).T)  # [C, C]
    bp = np.ascontiguousarray(b_p[:, None])

    xf = x.reshape(B, C, N)
    common = dict(
        wq_t=wq_t, bq=bq, wk_t=wk_t, bk=bk, wv_t=wv_t, bv=bv,
        wpe=wpe, bvpe=bvpe, wp_t=wp_t, bp=bp, x=xf,
    )

    in_maps = []
    for c in range(NCORES):
        r0 = c * RS
        xq = np.ascontiguousarray(xf[:, :, c * NS : (c + 1) * NS])
        xh = np.zeros((B, C, RS + 2, W), np.float32)
        lo, hi = max(r0 - 1, 0), min(r0 + RS + 1, H)
        xh[:, :, lo - (r0 - 1) : hi - (r0 - 1), :] = x[:, :, lo:hi, :]
        hmask = np.zeros((RS + 2, 66), np.float32)
        for ri in range(RS + 2):
            if 0 <= r0 - 1 + ri < H:
                hmask[ri, :] = 1.0
        m = dict(common)
        m["xq"] = xq
        m["xh"] = np.ascontiguousarray(xh.reshape(B, C, (RS + 2) * W))
        m["hmask"] = np.ascontiguousarray(
            np.broadcast_to(hmask.reshape(1, -1), (128, (RS + 2) * 66)).copy()
        )
        in_maps.append(m)
    return in_maps


def kernel(**inputs) -> np.ndarray:
    from concourse.bass_utils import run_bass_kernel_spmd

    if "nc" not in _CACHE:
        from concourse import mybir

        nc = build_module()
        # hw-only lowering fix; CoreSim/TimelineSim need the pristine module
        _split_multi_waits(nc, mybir)
        _CACHE["nc"] = nc
    nc = _CACHE["nc"]
    in_maps = _prep_host(inputs)
    res = run_bass_kernel_spmd(nc, in_maps, list(range(NCORES)))
    out = np.empty((B, C, N), np.float32)
    for c in range(NCORES):
        out[:, :, c * NS : (c + 1) * NS] = res.results[c]["y"]
    return out.reshape(B, C, H, W)
